# revision 1
# baseline (speedup 1.0000x reference)
"""Trainium2 Bass kernel for a GQA attention block (LuluAttention).

Problem: hidden_states [2, 2048, 2048], 16 q heads / 4 kv heads of dim 128,
RoPE, softmax attention, output projection.

Sharding: 8 cores = 2 (batch) x 4 (query-row blocks of 512 rows).
Each core computes the full K/V for its batch (all 4 kv heads), Q for its
512-row query slice (all 16 heads), RoPE, attention, and the output
projection for its row slice. The full output is assembled on the host by
pure concatenation (no collectives needed).

Device-side layout: everything is kept transposed ([head_dim, seq] with
head_dim on SBUF partitions):
  - QT/KT come straight out of matmul(lhsT=W_slice, rhs=hsT)
  - scores are computed transposed: scoresT = K @ Q^T
  - exp(scoresT) feeds the AV matmul directly (lhsT = V tile natural)
  - softmax denominator = ones128 @ expT (broadcast across partitions)
  - ctxT slices are directly the lhsT for the output projection
so no on-device transposes are needed anywhere.  hs^T is prepared on the
host as part of input sharding.
"""

import os
import sys

if "/opt/trn_rl_repo" not in sys.path:
    sys.path.insert(0, "/opt/trn_rl_repo")

import numpy as np

B, S, H = 2, 2048, 2048
NH, NKV, D = 16, 4, 128
SQ = 512          # query rows per core
NCORES = 8
P = 128
NT = H // P       # 16 contraction tiles over hidden dim
ROPE_THETA = 10000.0
SCALE = 1.0 / float(np.sqrt(D))


def _rope_tables_T():
    """cosT/ssinT [D, S]: transposed RoPE tables with the rotate-half sign
    folded into ssin (negative for d<64)."""
    inv_freq = 1.0 / (ROPE_THETA ** (np.arange(0, D, 2, dtype=np.float64) / D))
    t = np.arange(S, dtype=np.float64)
    freqs = np.outer(t, inv_freq)                     # [S, D/2]
    emb = np.concatenate([freqs, freqs], axis=-1)     # [S, D]
    cos = np.cos(emb).astype(np.float32)
    sin = np.sin(emb).astype(np.float32)
    ssin = sin.copy()
    ssin[:, : D // 2] *= -1.0
    return np.ascontiguousarray(cos.T), np.ascontiguousarray(ssin.T)


def _build_program():
    from concourse import bacc, mybir, tile

    F32 = mybir.dt.float32
    F32R = mybir.dt.float32r
    AF = mybir.ActivationFunctionType

    def r(ap):
        # Plain fp32 matmul: the BIR verifier in this deployment rejects
        # f32->f32r bitcasts of DMA-written tiles ("not rounded to FP32r").
        return ap

    nc = bacc.Bacc(
        "TRN2", target_bir_lowering=False, debug=False, num_devices=NCORES
    )

    # f32r end-to-end for the projection operands: DMA'd f32r tiles are
    # legal FP32r matmul inputs (4x PE rate vs fp32), and f32r's numpy
    # binding is still float32 so the host side is unchanged.
    hsT = nc.dram_tensor("hsT", [H, S], F32R, kind="ExternalInput").ap()
    hsQ = nc.dram_tensor("hsQ", [H, SQ], F32R, kind="ExternalInput").ap()
    wq = nc.dram_tensor("wq", [H, NH * D], F32R, kind="ExternalInput").ap()
    wk = nc.dram_tensor("wk", [H, NKV * D], F32R, kind="ExternalInput").ap()
    wv = nc.dram_tensor("wv", [H, NKV * D], F32R, kind="ExternalInput").ap()
    wo = nc.dram_tensor("wo", [NH * D, H], F32, kind="ExternalInput").ap()
    bqT = nc.dram_tensor("bqT", [D, NH], F32, kind="ExternalInput").ap()
    bkT = nc.dram_tensor("bkT", [D, NKV], F32, kind="ExternalInput").ap()
    bv = nc.dram_tensor("bv", [1, NKV * D], F32, kind="ExternalInput").ap()
    cosq = nc.dram_tensor("cosq", [D, SQ], F32, kind="ExternalInput").ap()
    ssinq = nc.dram_tensor("ssinq", [D, SQ], F32, kind="ExternalInput").ap()
    cosk = nc.dram_tensor("cosk", [D, S], F32, kind="ExternalInput").ap()
    ssink = nc.dram_tensor("ssink", [D, S], F32, kind="ExternalInput").ap()
    out = nc.dram_tensor("out", [SQ, H], F32, kind="ExternalOutput").ap()

    with tile.TileContext(nc) as tc:
        # ---- long-lived pools (explicit alloc/release for phase-scoped
        # lifetimes that don't nest cleanly) ----
        # left side: long-lived (cst, kvp, qp); right side: phase scratch.
        # Each side is a stack — pools must be released in LIFO order.
        cst = tc.alloc_tile_pool(name="cst", bufs=1)
        kvp = tc.alloc_tile_pool(name="kvp", bufs=1)     # kt + vt (K..phase2)
        rp = tc.alloc_tile_pool(name="rp", bufs=2, side="right")  # rope scratch

        ones1 = cst.tile([1, P], F32, tag="ones1")
        nc.gpsimd.memset(ones1[:], 1.0)
        ones128 = cst.tile([P, P], F32, tag="ones128")
        nc.gpsimd.memset(ones128[:], 1.0)
        bqT_sb = cst.tile([D, NH], F32, tag="bqT")
        nc.sync.dma_start(bqT_sb[:], bqT[:, :])
        bkT_sb = cst.tile([D, NKV], F32, tag="bkT")
        nc.sync.dma_start(bkT_sb[:], bkT[:, :])
        bv_sb = cst.tile([1, NKV * D], F32, tag="bv")
        nc.sync.dma_start(bv_sb[:], bv[:, :])

        kt = [res_t for res_t in (
            kvp.tile([D, S], F32, tag=f"kt{g}", name=f"kt{g}")
            for g in range(NKV)
        )]
        vt = [res_t for res_t in (
            kvp.tile([P, NKV * D], F32, tag=f"v{t}", name=f"v{t}")
            for t in range(S // P)
        )]

        def rope(dst, tbl_cos, tbl_sin, width):
            """In-place RoPE on dst [128, width] (transposed layout)."""
            sh = rp.tile([P, 512], F32, tag="sh", name="sh")
            nc.sync.dma_start(sh[0 : D // 2, :width], dst[D // 2 : D, :])
            nc.sync.dma_start(sh[D // 2 : D, :width], dst[0 : D // 2, :])
            t1 = rp.tile([P, 512], F32, tag="rt1", name="rt1")
            nc.vector.tensor_mul(t1[:, :width], sh[:, :width], tbl_sin)
            t2 = rp.tile([P, 512], F32, tag="rt2", name="rt2")
            nc.vector.tensor_mul(t2[:, :width], dst[:], tbl_cos)
            nc.vector.tensor_add(dst[:], t1[:, :width], t2[:, :width])

        # ================= stage K =================
        # KT[g] [d=128, s2=2048] = (hs @ Wk + bk)^T, rope'd.
        # Two sweeps over hsT (g pairs) with 8 PSUM banks live each.
        pk = tc.alloc_tile_pool(name="ps_k", bufs=8, space="PSUM")
        wsk = tc.alloc_tile_pool(name="wsk", bufs=1, side="right")
        for sweep in range(2):
            gs = (2 * sweep, 2 * sweep + 1)
            banks = {
                (g, c): pk.tile([P, 512], F32, tag="pj", name=f"pk_{g}_{c}")
                for g in gs
                for c in range(4)
            }
            for ht in range(NT):
                hst = wsk.tile([P, S], F32R, tag="hs", bufs=3, name="hst")
                nc.sync.dma_start(hst[:], hsT[ht * P : (ht + 1) * P, :])
                for g in gs:
                    wkt = wsk.tile([P, P], F32R, tag="wk", bufs=4, name="wkt")
                    nc.sync.dma_start(
                        wkt[:],
                        wk[ht * P : (ht + 1) * P, g * D : (g + 1) * D],
                    )
                    for c in range(4):
                        nc.tensor.matmul(
                            banks[(g, c)][:],
                            r(wkt[:]),
                            r(hst[:, c * 512 : (c + 1) * 512]),
                            start=(ht == 0),
                            stop=(ht == NT - 1),
                        )
            for g in gs:
                for c in range(4):
                    # copy + bias (bk varies along partitions here)
                    nc.scalar.activation(
                        kt[g][:, c * 512 : (c + 1) * 512],
                        banks[(g, c)][:],
                        AF.Identity,
                        bias=bkT_sb[:, g : g + 1],
                    )
            # rope per 512-chunk; share table tiles across g
            for c in range(4):
                ck = wsk.tile([P, 512], F32, tag="tbc", bufs=2, name="ck")
                nc.sync.dma_start(ck[:], cosk[:, c * 512 : (c + 1) * 512])
                sk = wsk.tile([P, 512], F32, tag="tbs", bufs=2, name="sk")
                nc.sync.dma_start(sk[:], ssink[:, c * 512 : (c + 1) * 512])
                for g in gs:
                    rope(kt[g][:, c * 512 : (c + 1) * 512], ck[:], sk[:], 512)
        wsk.release()

        # ================= stage V =================
        # V[t] [s2-tile=128, 4*128] = hs @ Wv + bv (natural layout).
        wvp = tc.alloc_tile_pool(name="wvp", bufs=1, side="right")
        wsv = tc.alloc_tile_pool(name="wsv", bufs=4, side="right")
        wvres = [
            wvp.tile([P, NKV * D], F32R, tag=f"wv{ht}", name=f"wv{ht}")
            for ht in range(NT)
        ]
        for ht in range(NT):
            nc.sync.dma_start(wvres[ht][:], wv[ht * P : (ht + 1) * P, :])
        for t in range(S // P):
            bank = pk.tile([P, 512], F32, tag="pj", name=f"pv_{t}")
            for ht in range(NT):
                hsl = wsv.tile([P, P], F32R, tag="hsv", name="hsl")
                nc.sync.dma_start(
                    hsl[:],
                    hsT[ht * P : (ht + 1) * P, t * P : (t + 1) * P],
                )
                nc.tensor.matmul(
                    bank[:],
                    r(hsl[:]),
                    r(wvres[ht][:]),
                    start=(ht == 0),
                    stop=False,
                )
            nc.tensor.matmul(
                bank[:], r(ones1[:]), r(bv_sb[:]), start=False, stop=True
            )
            nc.scalar.copy(vt[t][:], bank[:])
        wsv.release()
        wvp.release()

        # ================= stage Q =================
        # QT[h] [d=128, s1=512] = (hs_q @ Wq + bq)^T, rope'd.
        qp = tc.alloc_tile_pool(name="qp", bufs=1)       # qt (Q..phase2)
        hsqp = tc.alloc_tile_pool(name="hsqp", bufs=1, side="right")
        wsq = tc.alloc_tile_pool(name="wsq", bufs=4, side="right")
        qt = [
            qp.tile([D, SQ], F32, tag=f"qt{h}", name=f"qt{h}")
            for h in range(NH)
        ]
        cosq_sb = hsqp.tile([D, SQ], F32, tag="cosq")
        nc.sync.dma_start(cosq_sb[:], cosq[:, :])
        ssinq_sb = hsqp.tile([D, SQ], F32, tag="ssinq")
        nc.sync.dma_start(ssinq_sb[:], ssinq[:, :])
        hsqres = [
            hsqp.tile([P, SQ], F32R, tag=f"hsq{ht}", name=f"hsq{ht}")
            for ht in range(NT)
        ]
        for ht in range(NT):
            nc.sync.dma_start(hsqres[ht][:], hsQ[ht * P : (ht + 1) * P, :])
        for h in range(NH):
            bank = pk.tile([P, 512], F32, tag="pj", name=f"pq_{h}")
            for ht in range(NT):
                wqt = wsq.tile([P, P], F32R, tag="wq", name="wqt")
                nc.sync.dma_start(
                    wqt[:],
                    wq[ht * P : (ht + 1) * P, h * D : (h + 1) * D],
                )
                nc.tensor.matmul(
                    bank[:],
                    r(wqt[:]),
                    r(hsqres[ht][:]),
                    start=(ht == 0),
                    stop=(ht == NT - 1),
                )
            nc.scalar.activation(
                qt[h][:], bank[:], AF.Identity, bias=bqT_sb[:, h : h + 1]
            )
            rope(qt[h], cosq_sb[:], ssinq_sb[:], SQ)
        wsq.release()
        hsqp.release()
        rp.release()
        pk.release()

        # ================= phase 2: attention =================
        pa = tc.alloc_tile_pool(name="ps_a", bufs=5, space="PSUM")
        pc = tc.alloc_tile_pool(name="ps_c", bufs=3, space="PSUM")
        ctxp = tc.alloc_tile_pool(name="ctxp", bufs=1, side="right")
        wsa = tc.alloc_tile_pool(name="wsa", bufs=4, side="right")
        ctx = [
            ctxp.tile([D, SQ], F32, tag=f"ctx{h}", name=f"ctx{h}")
            for h in range(NH)
        ]
        for h in range(NH):
            g = h // (NH // NKV)
            ctx_ps = pc.tile([P, SQ], F32, tag="acc", name=f"ctxps{h}")
            den_ps = pc.tile([P, SQ], F32, tag="acc", name=f"denps{h}")
            for t in range(S // P):
                sc = pa.tile([P, SQ], F32, tag="sc", bufs=5, name="sc")
                nc.tensor.matmul(
                    sc[:],
                    r(kt[g][:, t * P : (t + 1) * P]),
                    r(qt[h][:]),
                    start=True,
                    stop=True,
                )
                at = wsa.tile([P, SQ], F32, tag="at", name="at")
                nc.scalar.activation(at[:], sc[:], AF.Exp, scale=SCALE)
                nc.tensor.matmul(
                    ctx_ps[:],
                    r(vt[t][:, g * D : (g + 1) * D]),
                    r(at[:]),
                    start=(t == 0),
                    stop=(t == S // P - 1),
                )
                nc.tensor.matmul(
                    den_ps[:],
                    r(ones128[:]),
                    r(at[:]),
                    start=(t == 0),
                    stop=(t == S // P - 1),
                )
            rc = wsa.tile([P, SQ], F32, tag="rc", bufs=2, name="rc")
            nc.vector.reciprocal(rc[:], den_ps[:])
            nc.vector.tensor_mul(ctx[h][:], ctx_ps[:], rc[:])
        wsa.release()
        qp.release()
        kvp.release()
        pc.release()
        pa.release()

        # ================= phase 3: output projection =================
        po = tc.alloc_tile_pool(name="ps_o", bufs=8, space="PSUM")
        wso = tc.alloc_tile_pool(name="wso", bufs=3, side="right")
        for hc in range(4):
            banks = [
                po.tile([P, 512], F32, tag="po", name=f"po_{hc}_{i}")
                for i in range(4)
            ]
            for h in range(NH):
                wot = wso.tile([P, 512], F32, tag="wo", name="wot")
                nc.sync.dma_start(
                    wot[:],
                    wo[h * D : (h + 1) * D, hc * 512 : (hc + 1) * 512],
                )
                for s1t in range(4):
                    nc.tensor.matmul(
                        banks[s1t][:],
                        r(ctx[h][:, s1t * P : (s1t + 1) * P]),
                        r(wot[:]),
                        start=(h == 0),
                        stop=(h == NH - 1),
                    )
            for s1t in range(4):
                ob = wso.tile([P, 512], F32, tag="ob", name="ob")
                nc.scalar.copy(ob[:], banks[s1t][:])
                nc.sync.dma_start(
                    out[s1t * P : (s1t + 1) * P, hc * 512 : (hc + 1) * 512],
                    ob[:],
                )
        wso.release()
        po.release()
        ctxp.release()
        cst.release()

    nc.compile()
    return nc


_PROGRAM_CACHE = {}


def _get_program():
    if "nc" not in _PROGRAM_CACHE:
        _PROGRAM_CACHE["nc"] = _build_program()
    return _PROGRAM_CACHE["nc"]


def _prepare_in_maps(hidden_states, Wq, bq, Wk, bk, Wv, bv, Wo):
    hidden_states = np.asarray(hidden_states, dtype=np.float32)
    Wq = np.asarray(Wq, dtype=np.float32)
    bq = np.asarray(bq, dtype=np.float32)
    Wk = np.asarray(Wk, dtype=np.float32)
    bk = np.asarray(bk, dtype=np.float32)
    Wv = np.asarray(Wv, dtype=np.float32)
    bv = np.asarray(bv, dtype=np.float32)
    Wo = np.asarray(Wo, dtype=np.float32)

    cosT, ssinT = _rope_tables_T()
    bqT_h = np.ascontiguousarray(bq.reshape(NH, D).T)    # [128, 16]
    bkT_h = np.ascontiguousarray(bk.reshape(NKV, D).T)   # [128, 4]
    bv_h = bv.reshape(1, NKV * D)

    hsT_b = [np.ascontiguousarray(hidden_states[b].T) for b in range(B)]

    in_maps = []
    for core in range(NCORES):
        b, tq = core // 4, core % 4
        qoff = tq * SQ
        in_maps.append(
            {
                "hsT": hsT_b[b],
                "hsQ": np.ascontiguousarray(hsT_b[b][:, qoff : qoff + SQ]),
                "wq": Wq,
                "wk": Wk,
                "wv": Wv,
                "wo": Wo,
                "bqT": bqT_h,
                "bkT": bkT_h,
                "bv": bv_h,
                "cosq": np.ascontiguousarray(cosT[:, qoff : qoff + SQ]),
                "ssinq": np.ascontiguousarray(ssinT[:, qoff : qoff + SQ]),
                "cosk": cosT,
                "ssink": ssinT,
            }
        )
    return in_maps


def kernel(hidden_states, Wq, bq, Wk, bk, Wv, bv, Wo):
    from concourse.bass_utils import run_bass_kernel_spmd

    in_maps = _prepare_in_maps(hidden_states, Wq, bq, Wk, bk, Wv, bv, Wo)
    nc = _get_program()
    res = run_bass_kernel_spmd(
        nc, in_maps, core_ids=list(range(NCORES)), trace=False
    )

    out_full = np.empty((B, S, H), dtype=np.float32)
    for core in range(NCORES):
        b, tq = core // 4, core % 4
        out_full[b, tq * SQ : (tq + 1) * SQ, :] = res.results[core]["out"]
    return out_full



# revision 10
# speedup vs baseline: 1.3198x; 1.3198x over previous
"""Trainium2 Bass kernel for a GQA attention block (LuluAttention).

Problem: hidden_states [2, 2048, 2048], 16 q heads / 4 kv heads of dim 128,
RoPE, softmax attention, output projection.

Sharding: 8 cores = 2 (batch) x 4 (query-row blocks of 512 rows).
Each core computes the full K/V for its batch (all 4 kv heads), Q for its
512-row query slice (all 16 heads), RoPE, attention, and the output
projection for its row slice. The full output is assembled on the host by
pure concatenation (no collectives needed).

All matmul operands are bf16 (1 PE cycle/row vs 4 for fp32); PSUM
accumulation stays fp32 and the final output is fp32. hs^T is kept
resident in SBUF (16 tiles of [128, 2048] bf16) so K and V read it from
SBUF instead of re-streaming HBM. Device-side layout is transposed
([head_dim, seq] with head_dim on SBUF partitions) throughout:
  - QT/KT come straight out of matmul(lhsT=W_slice, rhs=hsT)
  - scores are computed transposed: scoresT = K @ Q^T
  - exp(scoresT) (bf16) feeds the AV matmul directly (lhsT = V tile)
  - softmax denominator = ones128 @ expT (broadcast across partitions)
  - ctxT slices are directly the lhsT for the output projection
so no on-device transposes are needed anywhere.
"""

import os
import sys

if "/opt/trn_rl_repo" not in sys.path:
    sys.path.insert(0, "/opt/trn_rl_repo")

import numpy as np

B, S, H = 2, 2048, 2048
NH, NKV, D = 16, 4, 128
SQ = 512          # query rows per core
NCORES = 8
P = 128
NT = H // P       # 16 contraction tiles over hidden dim
ROPE_THETA = 10000.0
SCALE = 1.0 / float(np.sqrt(D))


def _rope_tables_T():
    """cosT/ssinT [D, S]: transposed RoPE tables with the rotate-half sign
    folded into ssin (negative for d<64)."""
    inv_freq = 1.0 / (ROPE_THETA ** (np.arange(0, D, 2, dtype=np.float64) / D))
    t = np.arange(S, dtype=np.float64)
    freqs = np.outer(t, inv_freq)                     # [S, D/2]
    emb = np.concatenate([freqs, freqs], axis=-1)     # [S, D]
    cos = np.cos(emb).astype(np.float32)
    sin = np.sin(emb).astype(np.float32)
    ssin = sin.copy()
    ssin[:, : D // 2] *= -1.0
    return np.ascontiguousarray(cos.T), np.ascontiguousarray(ssin.T)


def _build_program():
    from concourse import bacc, mybir, tile

    F32 = mybir.dt.float32
    BF16 = mybir.dt.bfloat16
    AF = mybir.ActivationFunctionType

    nc = bacc.Bacc(
        "TRN2", target_bir_lowering=False, debug=False, num_devices=NCORES
    )

    # Inputs are packed into few tensors — per-buffer dispatch marshaling
    # costs ~50us/iter through the axon PJRT path.
    #   wkv [H+1, 1024]: cols 0:512 Wk, 512:1024 Wv; row H cols 512:1024 bv
    #   bqk [D, 20]:     cols 0:16 bq^T, 16:20 bk^T
    #   tbl [D, 5120]:   cosk | ssink | cosq | ssinq
    hsT = nc.dram_tensor("hsT", [H, S], BF16, kind="ExternalInput").ap()
    hsQ = nc.dram_tensor("hsQ", [H, SQ], BF16, kind="ExternalInput").ap()
    wq = nc.dram_tensor("wq", [H, NH * D], BF16, kind="ExternalInput").ap()
    wkv = nc.dram_tensor(
        "wkv", [H + 1, 2 * NKV * D], BF16, kind="ExternalInput"
    ).ap()
    wo = nc.dram_tensor("wo", [NH * D, H], BF16, kind="ExternalInput").ap()
    bqk = nc.dram_tensor("bqk", [D, NH + NKV], F32, kind="ExternalInput").ap()
    tbl_d = nc.dram_tensor(
        "tbl", [D, 2 * S + 2 * SQ], F32, kind="ExternalInput"
    ).ap()
    out = nc.dram_tensor("out", [SQ, H], F32, kind="ExternalOutput").ap()

    with tile.TileContext(nc) as tc:
        # ---- long-lived pools ----
        # left: constants + phase-2 operands; right: phase-scoped scratch
        # (stack discipline — release in LIFO order).
        cst = tc.alloc_tile_pool(name="cst", bufs=1)
        big = tc.alloc_tile_pool(name="big", bufs=1)

        ones1b = cst.tile([1, P], BF16, tag="ones1")
        nc.gpsimd.memset(ones1b[:], 1.0)
        ones128b = cst.tile([P, P], BF16, tag="ones128")
        nc.gpsimd.memset(ones128b[:], 1.0)
        bqk_sb = cst.tile([D, NH + NKV], F32, tag="bqk")
        nc.sync.dma_start(bqk_sb[:], bqk[:, :])
        bvb_sb = cst.tile([1, NKV * D], BF16, tag="bvb")
        nc.sync.dma_start(bvb_sb[:], wkv[H : H + 1, NKV * D : 2 * NKV * D])
        cosq_sb = cst.tile([D, SQ], F32, tag="cosq")
        nc.sync.dma_start(cosq_sb[:], tbl_d[:, 2 * S : 2 * S + SQ])
        ssinq_sb = cst.tile([D, SQ], F32, tag="ssinq")
        nc.sync.dma_start(ssinq_sb[:], tbl_d[:, 2 * S + SQ : 2 * S + 2 * SQ])

        kt = [big.tile([D, S], BF16, tag=f"kt{g}", name=f"kt{g}")
              for g in range(NKV)]
        vt = [big.tile([P, NKV * D], BF16, tag=f"v{t}", name=f"v{t}")
              for t in range(S // P)]
        qt = [big.tile([D, SQ], BF16, tag=f"qt{h}", name=f"qt{h}")
              for h in range(NH)]
        ctx = [big.tile([D, SQ], BF16, tag=f"ctx{h}", name=f"ctx{h}")
               for h in range(NH)]

        # right stack: RoPE tables, then resident hs^T
        tbl = tc.alloc_tile_pool(name="tbl", bufs=1, side="right")
        cosk_sb = tbl.tile([D, S], F32, tag="cosk")
        nc.sync.dma_start(cosk_sb[:], tbl_d[:, 0:S])
        ssink_sb = tbl.tile([D, S], F32, tag="ssink")
        nc.sync.dma_start(ssink_sb[:], tbl_d[:, S : 2 * S])

        hsp = tc.alloc_tile_pool(name="hsp", bufs=1, side="right")
        hs = [hsp.tile([P, S], BF16, tag=f"hs{ht}", name=f"hs{ht}")
              for ht in range(NT)]
        for ht in range(NT):
            nc.sync.dma_start(hs[ht][:], hsT[ht * P : (ht + 1) * P, :])

        psK = tc.alloc_tile_pool(name="ps_k", bufs=8, space="PSUM")

        def rope_into(dst, src, tbl_cos, tbl_sin, pool, width):
            """dst (bf16 slice) = rope(src [128, width] f32)."""
            sh = pool.tile([P, 512], F32, tag="sh", bufs=2, name="sh")
            nc.sync.dma_start(sh[0 : D // 2, :width], src[D // 2 : D, :width])
            nc.sync.dma_start(sh[D // 2 : D, :width], src[0 : D // 2, :width])
            t1 = pool.tile([P, 512], F32, tag="rt1", bufs=2, name="rt1")
            nc.vector.tensor_mul(t1[:, :width], sh[:, :width], tbl_sin)
            t2 = pool.tile([P, 512], F32, tag="rt2", bufs=2, name="rt2")
            nc.vector.tensor_mul(t2[:, :width], src[:, :width], tbl_cos)
            nc.vector.tensor_add(dst, t1[:, :width], t2[:, :width])

        # ================= stage K =================
        # KT[g] [d=128, s2=2048] = (hs @ Wk + bk)^T, rope'd, bf16.
        krp = tc.alloc_tile_pool(name="krp", bufs=2, side="right")
        for sweep in range(2):
            gs = (2 * sweep, 2 * sweep + 1)
            banks = {
                (g, c): psK.tile([P, 512], F32, tag="pj", name=f"pk_{g}_{c}")
                for g in gs
                for c in range(4)
            }
            for ht in range(NT):
                wkt = krp.tile([P, 2 * D], BF16, tag="wk", bufs=3, name="wkt")
                nc.sync.dma_start(
                    wkt[:],
                    wkv[ht * P : (ht + 1) * P, gs[0] * D : (gs[1] + 1) * D],
                )
                for gi, g in enumerate(gs):
                    for c in range(4):
                        nc.tensor.matmul(
                            banks[(g, c)][:],
                            wkt[:, gi * D : (gi + 1) * D],
                            hs[ht][:, c * 512 : (c + 1) * 512],
                            start=(ht == 0),
                            stop=(ht == NT - 1),
                        )
            for g in gs:
                for c in range(4):
                    tmp = krp.tile([P, 512], F32, tag="ktmp", bufs=2, name="ktmp")
                    nc.scalar.activation(
                        tmp[:], banks[(g, c)][:], AF.Identity,
                        bias=bqk_sb[:, NH + g : NH + g + 1],
                    )
                    rope_into(
                        kt[g][:, c * 512 : (c + 1) * 512], tmp[:],
                        cosk_sb[:, c * 512 : (c + 1) * 512],
                        ssink_sb[:, c * 512 : (c + 1) * 512],
                        krp, 512,
                    )
        krp.release()

        # ================= stage Q =================
        # QT[h] [d=128, s1=512] = (hs_q @ Wq + bq)^T, rope'd, bf16.
        # Emitted before stage V so Q's RoPE (ACT/DVE) overlaps V's matmuls.
        qrp = tc.alloc_tile_pool(name="qrp", bufs=2, side="right")
        for sweep in range(2):
            hset = range(8 * sweep, 8 * sweep + 8)
            qbank = {
                h: psK.tile([P, SQ], F32, tag="pj", name=f"pq_{h}")
                for h in hset
            }
            for ht in range(NT):
                hqt = qrp.tile([P, SQ], BF16, tag="hq", bufs=3, name="hqt")
                nc.sync.dma_start(hqt[:], hsQ[ht * P : (ht + 1) * P, :])
                wqt = qrp.tile([P, 8 * D], BF16, tag="wq", bufs=3, name="wqt")
                nc.sync.dma_start(
                    wqt[:],
                    wq[ht * P : (ht + 1) * P,
                       sweep * 8 * D : (sweep + 1) * 8 * D],
                )
                for i, h in enumerate(hset):
                    nc.tensor.matmul(
                        qbank[h][:],
                        wqt[:, i * D : (i + 1) * D],
                        hqt[:],
                        start=(ht == 0),
                        stop=(ht == NT - 1),
                    )
            for h in hset:
                tmp = qrp.tile([P, 512], F32, tag="qtmp", bufs=2, name="qtmp")
                nc.scalar.activation(
                    tmp[:], qbank[h][:], AF.Identity,
                    bias=bqk_sb[:, h : h + 1],
                )
                rope_into(qt[h][:], tmp[:], cosq_sb[:], ssinq_sb[:], qrp, SQ)
        qrp.release()

        # ================= stage V =================
        # V[t] [s2-tile=128, 4*128] = hs @ Wv + bv (natural layout), bf16.
        vrp = tc.alloc_tile_pool(name="vrp", bufs=2, side="right")
        for sweep in range(2):
            ts = range(8 * sweep, 8 * sweep + 8)
            vbank = {
                t: psK.tile([P, NKV * D], F32, tag="pj", name=f"pv_{t}")
                for t in ts
            }
            for ht in range(NT):
                wvt = vrp.tile([P, NKV * D], BF16, tag="wv", bufs=3, name="wvt")
                nc.sync.dma_start(
                    wvt[:],
                    wkv[ht * P : (ht + 1) * P, NKV * D : 2 * NKV * D],
                )
                for t in ts:
                    nc.tensor.matmul(
                        vbank[t][:],
                        hs[ht][:, t * P : (t + 1) * P],
                        wvt[:],
                        start=(ht == 0),
                        stop=False,
                    )
            for t in ts:
                nc.tensor.matmul(
                    vbank[t][:], ones1b[:], bvb_sb[:], start=False, stop=True
                )
                nc.scalar.copy(vt[t][:], vbank[t][:])
        vrp.release()
        hsp.release()
        tbl.release()
        psK.release()

        # ================= phase 2: attention =================
        pa = tc.alloc_tile_pool(name="ps_a", bufs=5, space="PSUM")
        pc = tc.alloc_tile_pool(name="ps_c", bufs=3, space="PSUM")
        asp = tc.alloc_tile_pool(name="asp", bufs=2, side="right")
        NTT = S // P  # 16 key tiles

        for h in range(NH):
            g = h // (NH // NKV)
            ctx_ps = pc.tile([P, SQ], F32, tag="acc", name=f"ctxps{h}")
            den_ps = pc.tile([P, SQ], F32, tag="acc", name=f"denps{h}")
            at = {}

            def emit_sc(t):
                sc = pa.tile([P, SQ], F32, tag="sc", bufs=5, name="sc")
                nc.tensor.matmul(
                    sc[:],
                    kt[g][:, t * P : (t + 1) * P],
                    qt[h][:],
                    start=True,
                    stop=True,
                )
                a = asp.tile([P, SQ], BF16, tag="at", bufs=5, name="at")
                nc.scalar.activation(a[:], sc[:], AF.Exp, scale=SCALE)
                at[t] = a

            emit_sc(0)
            emit_sc(1)
            for t in range(NTT):
                if t + 2 < NTT:
                    emit_sc(t + 2)
                nc.tensor.matmul(
                    ctx_ps[:],
                    vt[t][:, g * D : (g + 1) * D],
                    at[t][:],
                    start=(t == 0),
                    stop=(t == NTT - 1),
                )
                nc.tensor.matmul(
                    den_ps[:],
                    ones128b[:],
                    at[t][:],
                    start=(t == 0),
                    stop=(t == NTT - 1),
                )
                del at[t]
            rc = asp.tile([P, SQ], F32, tag="rc", bufs=2, name="rc")
            nc.vector.reciprocal(rc[:], den_ps[:])
            nc.vector.tensor_mul(ctx[h][:], ctx_ps[:], rc[:])
        asp.release()
        pc.release()
        pa.release()

        # ================= phase 3: output projection =================
        po = tc.alloc_tile_pool(name="ps_o", bufs=8, space="PSUM")
        wso = tc.alloc_tile_pool(name="wso", bufs=4, side="right")
        for hc in range(4):
            banks = [
                po.tile([P, 512], F32, tag="po", name=f"po_{hc}_{i}")
                for i in range(4)
            ]
            for h in range(NH):
                wot = wso.tile([P, 512], BF16, tag="wo", bufs=4, name="wot")
                nc.sync.dma_start(
                    wot[:],
                    wo[h * D : (h + 1) * D, hc * 512 : (hc + 1) * 512],
                )
                for s1t in range(4):
                    nc.tensor.matmul(
                        banks[s1t][:],
                        ctx[h][:, s1t * P : (s1t + 1) * P],
                        wot[:],
                        start=(h == 0),
                        stop=(h == NH - 1),
                    )
            for s1t in range(4):
                ob = wso.tile([P, 512], F32, tag="ob", bufs=4, name="ob")
                nc.scalar.copy(ob[:], banks[s1t][:])
                nc.sync.dma_start(
                    out[s1t * P : (s1t + 1) * P, hc * 512 : (hc + 1) * 512],
                    ob[:],
                )
        wso.release()
        po.release()
        big.release()
        cst.release()

    nc.compile()
    return nc


_PROGRAM_CACHE = {}


def _get_program():
    if "nc" not in _PROGRAM_CACHE:
        _PROGRAM_CACHE["nc"] = _build_program()
    return _PROGRAM_CACHE["nc"]


def _prepare_in_maps(hidden_states, Wq, bq, Wk, bk, Wv, bv, Wo):
    import ml_dtypes

    BF = ml_dtypes.bfloat16

    hidden_states = np.asarray(hidden_states, dtype=np.float32)
    Wq = np.asarray(Wq, dtype=np.float32)
    bq = np.asarray(bq, dtype=np.float32)
    Wk = np.asarray(Wk, dtype=np.float32)
    bk = np.asarray(bk, dtype=np.float32)
    Wv = np.asarray(Wv, dtype=np.float32)
    bv = np.asarray(bv, dtype=np.float32)
    Wo = np.asarray(Wo, dtype=np.float32)

    cosT, ssinT = _rope_tables_T()
    bqk_h = np.ascontiguousarray(
        np.concatenate(
            [bq.reshape(NH, D).T, bk.reshape(NKV, D).T], axis=1
        )
    )  # [128, 20]

    wkv_h = np.zeros((H + 1, 2 * NKV * D), dtype=BF)
    wkv_h[:H, : NKV * D] = Wk.astype(BF)
    wkv_h[:H, NKV * D :] = Wv.astype(BF)
    wkv_h[H, NKV * D :] = bv.astype(BF)

    wq_h = Wq.astype(BF)
    wo_h = Wo.astype(BF)
    hsT_b = [np.ascontiguousarray(hidden_states[b].T).astype(BF)
             for b in range(B)]

    in_maps = []
    for core in range(NCORES):
        b, tq = core // 4, core % 4
        qoff = tq * SQ
        tbl_h = np.concatenate(
            [cosT, ssinT,
             cosT[:, qoff : qoff + SQ], ssinT[:, qoff : qoff + SQ]],
            axis=1,
        )
        in_maps.append(
            {
                "hsT": hsT_b[b],
                "hsQ": np.ascontiguousarray(hsT_b[b][:, qoff : qoff + SQ]),
                "wq": wq_h,
                "wkv": wkv_h,
                "wo": wo_h,
                "bqk": bqk_h,
                "tbl": np.ascontiguousarray(tbl_h),
            }
        )
    return in_maps


def kernel(hidden_states, Wq, bq, Wk, bk, Wv, bv, Wo):
    from concourse.bass_utils import run_bass_kernel_spmd

    in_maps = _prepare_in_maps(hidden_states, Wq, bq, Wk, bk, Wv, bv, Wo)
    nc = _get_program()
    res = run_bass_kernel_spmd(
        nc, in_maps, core_ids=list(range(NCORES)), trace=False
    )

    out_full = np.empty((B, S, H), dtype=np.float32)
    for core in range(NCORES):
        b, tq = core // 4, core % 4
        out_full[b, tq * SQ : (tq + 1) * SQ, :] = res.results[core]["out"]
    return out_full
